# revision 1
# baseline (speedup 1.0000x reference)
"""ControlNorm1D online-normalization forward, Trainium2 Bass kernel.

Math (per feature column l, sequential over rows t):
    scale_t = sqrt(v_t + eps);  d_t = x_t - mu_t;  out_t = d_t / scale_t
    v_{t+1}  = a*v_t + a*(1-a)*d_t^2
    mu_{t+1} = a*mu_t + (1-a)*x_t

Both mu and v are first-order linear recurrences with constant decay, so blocks
of rows become matmuls against constant triangular coefficient matrices.

Blocking: rows are processed in pairs of 127-row blocks (254-row super-blocks).
For a pair with SBUF tiles R0/R1 ([128, 512]; partition 1+t = x row, partition 0
of R0 = virtual carry row -mu0):
    psD0 = LD_ev^T R0                 : partitions 1+t = d_t (t=0..126)
    psD1 = LD_x^T R0 + LD_od^T R1     : partitions 1+t = d_{127+t}; partition 0
                                        = -mu0_next (carry for the next pair)
    D2*  = psD*^2 (ScalarE Square); D20 partition 0 overwritten with v0
    psV0 = LV_ev^T D20
    psV1 = LV_x^T D20 + LV_od^T D21   : partition 0 = v0_next
    r*   = Rsqrt(psV* + eps);  out* = psD* * r*
Carries between pairs are two single-row PSUM->SBUF copies.

The feature dim L=4096 is sharded across 8 cores (512 each, no cross-core
communication).  Host-side, each core's x shard is PRE-TILED to the exact SBUF
layout [128 partitions, 65 blocks * 512] so device DMAs have one big contiguous
run per partition (26+ KB descriptors -> ~360 GB/s via SWDGE, measured; the
naive row-per-partition layout yields 2 KB descriptors at ~25-50 GB/s).
Outputs are written back over the same SBUF buffer and un-pretiled on host.
"""

import numpy as np

AFWD = 0.999
EPS = 1e-5
N_ROWS = 8192
L_FULL = 4096
N_CORES = 8
LC = L_FULL // N_CORES  # 512 features per core
B = 127                 # rows per block (partition 1+t holds row t)
NBLK = 65               # 64 full blocks + 1 short (64 rows)
NPAIR = 32              # paired blocks; block 64 handled as a single tail
CB = 14                 # blocks per DMA chunk
USE_RAW_RSQRT = True    # Rsqrt LUT: ~4.4e-5 max rel err (measured); ln/exp: ~1.5e-6
USE_F32R = True         # float32r matmuls: 1-pass (vs fp32 2-pass), ~1.2e-4 input rounding

_f32 = np.float32


def _tri(me, ve):
    a = AFWD
    L = np.zeros((128, 128))
    for t in range(127):
        for s in range(127):
            if s == t:
                L[1 + s, 1 + t] += me
            if s < t:
                L[1 + s, 1 + t] += ve * a ** (t - 1 - s)
    return L


def _build_mats():
    a = AFWD
    LD_ev = _tri(1.0, -(1 - a))
    LV_ev = _tri(0.0, a * (1 - a))
    for t in range(127):
        LD_ev[0, 1 + t] = a**t
        LV_ev[0, 1 + t] = a**t
    LD_x = np.zeros((128, 128))
    LV_x = np.zeros((128, 128))
    for t in range(127):
        LD_x[0, 1 + t] = a ** (127 + t)
        LV_x[0, 1 + t] = a ** (127 + t)
        for s in range(127):
            LD_x[1 + s, 1 + t] = -(1 - a) * a ** (127 + t - 1 - s)
            LV_x[1 + s, 1 + t] = a * (1 - a) * a ** (127 + t - 1 - s)
    LD_x[0, 0] = a**254
    LV_x[0, 0] = a**254
    for s in range(127):
        LD_x[1 + s, 0] = -(1 - a) * a ** (253 - s)
        LV_x[1 + s, 0] = a * (1 - a) * a ** (253 - s)
    LD_od = _tri(1.0, -(1 - a))
    LV_od = _tri(0.0, a * (1 - a))
    for s in range(127):
        LD_od[1 + s, 0] = -(1 - a) * a ** (126 - s)
        LV_od[1 + s, 0] = a * (1 - a) * a ** (126 - s)
    LD_s = _tri(1.0, -(1 - a))
    LV_s = _tri(0.0, a * (1 - a))
    for t in range(127):
        LD_s[0, 1 + t] = a**t
        LV_s[0, 1 + t] = a**t
    mats = [LD_ev, LD_x, LD_od, LV_ev, LV_x, LV_od, LD_s, LV_s]
    return np.stack([m.astype(_f32) for m in mats])  # [8, 128, 128]


def _pretile(x_c, m_c):
    """[8192, LC] -> [128, NBLK*LC]: partition 1+t of block-slice i = row i*127+t.
    Partition 0 of block 0 carries the virtual row -m."""
    xp = np.zeros((128, NBLK * LC), _f32)
    full = x_c[: 64 * B].reshape(64, B, LC).transpose(1, 0, 2)  # [127, 64, LC]
    xp[1:128, : 64 * LC] = full.reshape(B, 64 * LC)
    xp[1:65, 64 * LC :] = x_c[64 * B :]
    xp[0, :LC] = -m_c
    return xp


def _unpretile(op):
    """Inverse of _pretile for the output buffer."""
    out = np.empty((N_ROWS, LC), _f32)
    out[: 64 * B] = (
        op[1:128, : 64 * LC].reshape(B, 64, LC).transpose(1, 0, 2).reshape(-1, LC)
    )
    out[64 * B :] = op[1:65, 64 * LC :]
    return out


_PROGRAM_CACHE: dict = {}


def _raw_act(eng, out, in_, func, bias_ap, scale, mybir):
    ins = [
        eng.lower_ap(in_),
        eng.lower_ap(bias_ap),
        mybir.ImmediateValue(dtype=mybir.dt.float32, value=float(scale)),
        mybir.ImmediateValue(dtype=mybir.dt.float32, value=0.0),
    ]
    return eng.add_instruction(
        mybir.InstActivation(
            name=eng.bass.get_next_instruction_name(),
            func=func,
            ins=ins,
            outs=[eng.lower_ap(out)],
        )
    )


def _build_program():
    if "nc" in _PROGRAM_CACHE:
        return _PROGRAM_CACHE["nc"]

    import concourse.bacc as bacc
    import concourse.tile as tile
    from concourse import mybir

    nc = bacc.Bacc(
        "TRN2",
        target_bir_lowering=False,
        debug=False,
        enable_asserts=False,
        num_devices=N_CORES,
    )
    f32 = mybir.dt.float32
    mmdt = mybir.dt.float32r if USE_F32R else f32

    xp_d = nc.dram_tensor("xp", [128, NBLK * LC], f32, kind="ExternalInput").ap()
    var_d = nc.dram_tensor("var", [LC], f32, kind="ExternalInput").ap()
    mats_d = nc.dram_tensor("mats", [8, 128, 128], f32, kind="ExternalInput").ap()
    op_d = nc.dram_tensor("op", [128, NBLK * LC], f32, kind="ExternalOutput").ap()

    chunks = []
    b0 = 0
    while b0 < NBLK:
        b1 = min(b0 + CB, NBLK)
        chunks.append((b0, b1))
        b0 = b1

    with tile.TileContext(nc) as tc:
        with (
            tc.tile_pool(name="consts", bufs=1) as consts,
            tc.tile_pool(name="work", bufs=1) as work,
            tc.tile_pool(name="d2", bufs=4) as d2_pool,
            tc.tile_pool(name="rs", bufs=4) as r_pool,
            tc.tile_pool(name="psD", bufs=2, space="PSUM") as psD_pool,
            tc.tile_pool(name="psV", bufs=2, space="PSUM") as psV_pool,
        ):
            mat_tiles = []
            for mi in range(8):
                mt = consts.tile([128, 128], mmdt, tag=f"mat{mi}")
                nc.gpsimd.dma_start(out=mt[:], in_=mats_d[mi, :, :])
                mat_tiles.append(mt)
            eps_t = consts.tile([128, 1], f32)
            nc.vector.memset(eps_t[:], EPS)

            W = work.tile([128, NBLK * LC], mmdt)

            # chunked SWDGE loads (big contiguous per-partition descriptors)
            for (b0, b1) in chunks:
                nc.gpsimd.dma_start(
                    out=W[:, b0 * LC : b1 * LC], in_=xp_d[:, b0 * LC : b1 * LC]
                )

            # block-0 virtual row (-m) is folded into the pretiled xp host-side
            v0_t = consts.tile([1, LC], f32)
            nc.sync.dma_start(out=v0_t[:], in_=var_d[None, :])

            LD_ev, LD_x, LD_od, LV_ev, LV_x, LV_od, LD_s, LV_s = (
                mt[:] for mt in mat_tiles
            )

            prev_D1 = None
            prev_V1 = None
            out_chunk_done = [False] * len(chunks)

            def emit_out_dma(upto_block):
                # fire the out-DMA for any chunk fully computed
                for ci, (b0, b1) in enumerate(chunks):
                    if not out_chunk_done[ci] and b1 <= upto_block:
                        nc.gpsimd.dma_start(
                            out=op_d[:, b0 * LC : b1 * LC],
                            in_=W[:, b0 * LC : b1 * LC],
                        )
                        out_chunk_done[ci] = True

            def rsqrt_to(r, psV, rows=128):
                if USE_RAW_RSQRT:
                    _raw_act(
                        nc.scalar,
                        r[0:rows, :],
                        psV[0:rows, :],
                        mybir.ActivationFunctionType.Rsqrt,
                        eps_t[0:rows, :],
                        1.0,
                        mybir,
                    )
                else:
                    nc.scalar.activation(
                        out=r[0:rows, :],
                        in_=psV[0:rows, :],
                        func=mybir.ActivationFunctionType.Ln,
                        bias=eps_t[0:rows, :],
                        scale=1.0,
                    )
                    nc.scalar.activation(
                        out=r[0:rows, :],
                        in_=r[0:rows, :],
                        func=mybir.ActivationFunctionType.Exp,
                        bias=0.0,
                        scale=-0.5,
                    )

            for i in range(NPAIR):
                s0 = slice(2 * i * LC, (2 * i + 1) * LC)
                s1 = slice((2 * i + 1) * LC, (2 * i + 2) * LC)
                R0 = W[:, s0]
                R1 = W[:, s1]

                if i > 0:
                    nc.vector.tensor_copy(out=W[0:1, s0], in_=prev_D1[0:1, :])

                psD0 = psD_pool.tile([128, LC], f32, tag="psD0")
                psD1 = psD_pool.tile([128, LC], f32, tag="psD1")
                nc.tensor.matmul(psD0[:], LD_ev, R0, start=True, stop=True)
                nc.tensor.matmul(psD1[:], LD_x, R0, start=True, stop=False)
                nc.tensor.matmul(psD1[:], LD_od, R1, start=False, stop=True)

                d20 = d2_pool.tile([128, LC], mmdt, tag="d20")
                d21 = d2_pool.tile([128, LC], mmdt, tag="d21")
                nc.scalar.square(out=d20[:, :], in_=psD0[:, :])
                if i == 0:
                    nc.vector.tensor_copy(out=d20[0:1, :], in_=v0_t[:])
                else:
                    nc.vector.tensor_copy(out=d20[0:1, :], in_=prev_V1[0:1, :])
                nc.scalar.square(out=d21[:, :], in_=psD1[:, :])

                psV0 = psV_pool.tile([128, LC], f32, tag="psV0")
                psV1 = psV_pool.tile([128, LC], f32, tag="psV1")
                nc.tensor.matmul(psV0[:], LV_ev, d20[:], start=True, stop=True)
                nc.tensor.matmul(psV1[:], LV_x, d20[:], start=True, stop=False)
                nc.tensor.matmul(psV1[:], LV_od, d21[:], start=False, stop=True)

                r0 = r_pool.tile([128, LC], f32, tag="r0")
                r1 = r_pool.tile([128, LC], f32, tag="r1")
                rsqrt_to(r0, psV0)
                rsqrt_to(r1, psV1)

                nc.vector.tensor_mul(out=W[:, s0], in0=psD0[:, :], in1=r0[:, :])
                nc.vector.tensor_mul(out=W[:, s1], in0=psD1[:, :], in1=r1[:, :])

                prev_D1 = psD1
                prev_V1 = psV1
                emit_out_dma(2 * i + 2)

            # tail: block 64 (64 rows; pretile zero-padded the rest)
            st = slice(64 * LC, 65 * LC)
            nc.vector.tensor_copy(out=W[0:1, st], in_=prev_D1[0:1, :])
            psDt = psD_pool.tile([128, LC], f32, tag="psD0")
            nc.tensor.matmul(psDt[:], LD_s, W[:, st], start=True, stop=True)
            d2t = d2_pool.tile([128, LC], mmdt, tag="d20")
            nc.scalar.square(out=d2t[:, :], in_=psDt[:, :])
            nc.vector.tensor_copy(out=d2t[0:1, :], in_=prev_V1[0:1, :])
            psVt = psV_pool.tile([128, LC], f32, tag="psV0")
            nc.tensor.matmul(psVt[:], LV_s, d2t[:], start=True, stop=True)
            rt = r_pool.tile([128, LC], f32, tag="r0")
            rsqrt_to(rt, psVt)
            nc.vector.tensor_mul(out=W[:, st], in0=psDt[:, :], in1=rt[:, :])
            emit_out_dma(NBLK)

    nc.compile()
    _PROGRAM_CACHE["nc"] = nc
    return nc


def kernel(x: np.ndarray, m: np.ndarray, var: np.ndarray) -> np.ndarray:
    from concourse.bass_utils import run_bass_kernel_spmd

    x = np.asarray(x, dtype=_f32)
    m = np.ascontiguousarray(np.asarray(m, dtype=_f32))
    var = np.ascontiguousarray(np.asarray(var, dtype=_f32))
    assert x.shape == (N_ROWS, L_FULL), x.shape

    nc = _build_program()
    mats = _build_mats()

    in_maps = []
    for c in range(N_CORES):
        sl = slice(c * LC, (c + 1) * LC)
        in_maps.append(
            {
                "xp": _pretile(np.ascontiguousarray(x[:, sl]), m[sl]),
                "var": np.ascontiguousarray(var[sl]),
                "mats": mats,
            }
        )

    res = run_bass_kernel_spmd(nc, in_maps, core_ids=list(range(N_CORES)))
    out = np.concatenate(
        [_unpretile(res.results[c]["op"]) for c in range(N_CORES)], axis=1
    )
    return out.astype(_f32, copy=False)



# revision 7
# speedup vs baseline: 1.8929x; 1.8929x over previous
"""ControlNorm1D online-normalization forward, Trainium2 Bass kernel.

Math (per feature column l, sequential over rows t):
    scale_t = sqrt(v_t + eps);  d_t = x_t - mu_t;  out_t = d_t / scale_t
    v_{t+1}  = a*v_t + a*(1-a)*d_t^2
    mu_{t+1} = a*mu_t + (1-a)*x_t

Both mu and v are first-order linear recurrences with constant decay, so blocks
of rows become matmuls against constant triangular coefficient matrices.

Blocking: rows are processed in pairs of 127-row blocks (254-row super-blocks).
For a pair with SBUF tiles R0/R1 ([128, 512]; partition 1+t = x row, partition 0
of R0 = virtual carry row -mu0):
    psD0 = LD_ev^T R0                 : partitions 1+t = d_t (t=0..126)
    psD1 = LD_x^T R0 + LD_od^T R1     : partitions 1+t = d_{127+t}; partition 0
                                        = -mu0_next (carry for the next pair)
    D2*  = psD*^2 (ScalarE Square); D20 partition 0 overwritten with v0
    psV0 = LV_ev^T D20
    psV1 = LV_x^T D20 + LV_od^T D21   : partition 0 = v0_next
    r*   = Rsqrt(psV* + eps);  out* = psD* * r*

Performance structure (v2): the naive per-pair emission order couples every
engine into one serial loop (copy -> matmul -> square -> copy -> matmul ->
rsqrt -> mul per pair, ~7.6us/pair measured).  This version software-pipelines
the emission: the V-side of pair i-1 (V matmuls, rsqrt, output muls) is
interleaved with the D-side of pair i, so the carry-chain latency hides under
TensorE work and TensorE (6 matmuls/pair, cost = 512 free-dim rows each) is
the pacing engine.  Matmul dtype is bf16: same 1 cycle/row PE speed as f32r,
but half the DMA bytes (input AND output), and half the SBUF footprint.

The feature dim L=4096 is sharded across 8 cores (512 each, no cross-core
communication).  Host-side, each core's x shard is PRE-TILED to the exact SBUF
layout [128 partitions, 65 blocks * 512] so device DMAs have one big contiguous
run per partition.  Outputs are written back over the same SBUF buffer (bf16)
and un-pretiled + upcast on host.
"""

import numpy as np
import ml_dtypes

AFWD = 0.999
EPS = 1e-5
N_ROWS = 8192
L_FULL = 4096
N_CORES = 8
LC = L_FULL // N_CORES  # 512 features per core
B = 127                 # rows per block (partition 1+t holds row t)
NBLK = 65               # 64 full blocks + 1 short (64 rows)
NPAIR = 32              # paired blocks; block 64 handled as a single tail

_f32 = np.float32
_bf16 = ml_dtypes.bfloat16

# input-DMA chunk boundaries (blocks): small first chunks so compute can start
IN_CHUNKS = [(0, 4), (4, 12), (12, 26), (26, 40), (40, 54), (54, 65)]
# output-DMA chunk boundaries (blocks)
OUT_CHUNKS = [(0, 14), (14, 28), (28, 42), (42, 56), (56, 65)]


def _tri(me, ve):
    a = AFWD
    L = np.zeros((128, 128))
    for t in range(127):
        for s in range(127):
            if s == t:
                L[1 + s, 1 + t] += me
            if s < t:
                L[1 + s, 1 + t] += ve * a ** (t - 1 - s)
    return L


def _build_mats():
    a = AFWD
    LD_ev = _tri(1.0, -(1 - a))
    LV_ev = _tri(0.0, a * (1 - a))
    for t in range(127):
        LD_ev[0, 1 + t] = a**t
        LV_ev[0, 1 + t] = a**t
    LD_x = np.zeros((128, 128))
    LV_x = np.zeros((128, 128))
    for t in range(127):
        LD_x[0, 1 + t] = a ** (127 + t)
        LV_x[0, 1 + t] = a ** (127 + t)
        for s in range(127):
            LD_x[1 + s, 1 + t] = -(1 - a) * a ** (127 + t - 1 - s)
            LV_x[1 + s, 1 + t] = a * (1 - a) * a ** (127 + t - 1 - s)
    LD_x[0, 0] = a**254
    LV_x[0, 0] = a**254
    for s in range(127):
        LD_x[1 + s, 0] = -(1 - a) * a ** (253 - s)
        LV_x[1 + s, 0] = a * (1 - a) * a ** (253 - s)
    LD_od = _tri(1.0, -(1 - a))
    LV_od = _tri(0.0, a * (1 - a))
    for s in range(127):
        LD_od[1 + s, 0] = -(1 - a) * a ** (126 - s)
        LV_od[1 + s, 0] = a * (1 - a) * a ** (126 - s)
    mats = [LD_ev, LD_x, LD_od, LV_ev, LV_x, LV_od]
    return np.stack([m.astype(_bf16) for m in mats])  # [6, 128, 128] bf16


def _pretile(x_c, m_c):
    """[8192, LC] f32 -> [128, NBLK*LC] bf16: partition 1+t of block-slice i =
    row i*127+t.  Partition 0 of block 0 carries the virtual row -m."""
    xp = np.zeros((128, NBLK * LC), _bf16)
    xb = x_c.astype(_bf16)
    full = xb[: 64 * B].reshape(64, B, LC).transpose(1, 0, 2)  # [127, 64, LC]
    xp[1:128, : 64 * LC] = full.reshape(B, 64 * LC)
    xp[1:65, 64 * LC :] = xb[64 * B :]
    xp[0, :LC] = (-m_c).astype(_bf16)
    return xp


def _unpretile(op):
    """Inverse of _pretile for the (bf16) output buffer -> f32."""
    opf = np.asarray(op).astype(_f32)
    out = np.empty((N_ROWS, LC), _f32)
    out[: 64 * B] = (
        opf[1:128, : 64 * LC].reshape(B, 64, LC).transpose(1, 0, 2).reshape(-1, LC)
    )
    out[64 * B :] = opf[1:65, 64 * LC :]
    return out


_PROGRAM_CACHE: dict = {}


def _raw_act(eng, out, in_, func, bias_ap, scale, mybir):
    ins = [
        eng.lower_ap(in_),
        eng.lower_ap(bias_ap),
        mybir.ImmediateValue(dtype=mybir.dt.float32, value=float(scale)),
        mybir.ImmediateValue(dtype=mybir.dt.float32, value=0.0),
    ]
    return eng.add_instruction(
        mybir.InstActivation(
            name=eng.bass.get_next_instruction_name(),
            func=func,
            ins=ins,
            outs=[eng.lower_ap(out)],
        )
    )


def _build_program():
    if "nc" in _PROGRAM_CACHE:
        return _PROGRAM_CACHE["nc"]

    import concourse.bacc as bacc
    import concourse.tile as tile
    from concourse import mybir

    nc = bacc.Bacc(
        "TRN2",
        target_bir_lowering=False,
        debug=False,
        enable_asserts=False,
        num_devices=N_CORES,
    )
    f32 = mybir.dt.float32
    bf16 = mybir.dt.bfloat16

    xp_d = nc.dram_tensor("xp", [128, NBLK * LC], bf16, kind="ExternalInput").ap()
    var_d = nc.dram_tensor("var", [LC], f32, kind="ExternalInput").ap()
    mats_d = nc.dram_tensor("mats", [6, 128, 128], bf16, kind="ExternalInput").ap()
    op_d = nc.dram_tensor("op", [128, NBLK * LC], bf16, kind="ExternalOutput").ap()

    with tile.TileContext(nc) as tc:
        with (
            tc.tile_pool(name="consts", bufs=1) as consts,
            tc.tile_pool(name="work", bufs=1) as work,
            tc.tile_pool(name="d2", bufs=2) as d2_pool,
            tc.tile_pool(name="rs", bufs=2) as r_pool,
            tc.tile_pool(name="psD", bufs=2, space="PSUM") as psD_pool,
            tc.tile_pool(name="psV", bufs=1, space="PSUM") as psV_pool,
        ):
            mat_tiles = []
            for mi in range(6):
                mt = consts.tile([128, 128], bf16, tag=f"mat{mi}")
                nc.gpsimd.dma_start(out=mt[:], in_=mats_d[mi, :, :])
                mat_tiles.append(mt)
            eps_t = consts.tile([128, 1], f32)
            nc.vector.memset(eps_t[:], EPS)
            warm = consts.tile([1, 1], f32)

            W = work.tile([128, NBLK * LC], bf16)

            # chunked SWDGE loads (big contiguous per-partition descriptors)
            for (b0, b1) in IN_CHUNKS:
                nc.gpsimd.dma_start(
                    out=W[:, b0 * LC : b1 * LC], in_=xp_d[:, b0 * LC : b1 * LC]
                )

            v0_t = consts.tile([1, LC], f32)
            nc.sync.dma_start(out=v0_t[:], in_=var_d[None, :])

            LD_ev, LD_x, LD_od, LV_ev, LV_x, LV_od = (mt[:] for mt in mat_tiles)

            # warm the ScalarE activation tables (Square / Rsqrt) during the
            # input-DMA ramp so the first real pair doesn't eat ACT_TABLE_LOAD
            nc.scalar.square(out=warm[:], in_=eps_t[0:1, 0:1])
            _raw_act(
                nc.scalar, warm[:], eps_t[0:1, 0:1],
                mybir.ActivationFunctionType.Rsqrt, eps_t[0:1, 0:1], 1.0, mybir,
            )

            def rsqrt_to(r, psV):
                _raw_act(
                    nc.scalar, r[:, :], psV[:, :],
                    mybir.ActivationFunctionType.Rsqrt, eps_t[:, :], 1.0, mybir,
                )

            out_chunk_done = [False] * len(OUT_CHUNKS)

            def emit_out_dma(upto_block):
                for ci, (b0, b1) in enumerate(OUT_CHUNKS):
                    if not out_chunk_done[ci] and b1 <= upto_block:
                        nc.gpsimd.dma_start(
                            out=op_d[:, b0 * LC : b1 * LC],
                            in_=W[:, b0 * LC : b1 * LC],
                        )
                        out_chunk_done[ci] = True

            # pipeline state from pair i-1
            prev = None  # dict with psD0, psD1, psV0, psV1, d20, d21, r0, r1, s0, s1

            for i in range(NPAIR):
                s0 = slice(2 * i * LC, (2 * i + 1) * LC)
                s1 = slice((2 * i + 1) * LC, (2 * i + 2) * LC)

                cur = {
                    "s0": s0,
                    "s1": s1,
                    "psD0": psD_pool.tile([128, LC], f32, tag="psD0", name=f"psD0_{i}"),
                    "psD1": psD_pool.tile([128, LC], f32, tag="psD1", name=f"psD1_{i}", bufs=3),
                    "psV0": psV_pool.tile([128, LC], f32, tag="psV0", name=f"psV0_{i}", bufs=1),
                    "psV1": psV_pool.tile([128, LC], f32, tag="psV1", name=f"psV1_{i}", bufs=2),
                    "d20": d2_pool.tile([128, LC], bf16, tag="d20", name=f"d20_{i}"),
                    "d21": d2_pool.tile([128, LC], bf16, tag="d21", name=f"d21_{i}"),
                    "r0": r_pool.tile([128, LC], f32, tag="r0", name=f"r0_{i}"),
                    "r1": r_pool.tile([128, LC], f32, tag="r1", name=f"r1_{i}"),
                }

                # ---- TensorE slot: carry-free od matmul first --------------
                nc.tensor.matmul(cur["psD1"][:], LD_od, W[:, s1], start=True, stop=False)
                if i > 0:
                    # V matmuls of pair i-1 (d2/carry produced last slot)
                    nc.tensor.matmul(prev["psV1"][:], LV_od, prev["d21"][:], start=True, stop=False)
                    nc.tensor.matmul(prev["psV1"][:], LV_x, prev["d20"][:], start=False, stop=True)
                    nc.tensor.matmul(prev["psV0"][:], LV_ev, prev["d20"][:], start=True, stop=True)
                # carry-dependent D matmuls (copyD(i) ran mid-previous-slot)
                nc.tensor.matmul(cur["psD0"][:], LD_ev, W[:, s0], start=True, stop=True)
                nc.tensor.matmul(cur["psD1"][:], LD_x, W[:, s0], start=False, stop=True)

                # ---- Scalar queue: rsqrt(i-1) then squares(i) --------------
                if i > 0:
                    rsqrt_to(prev["r0"], prev["psV0"])
                    rsqrt_to(prev["r1"], prev["psV1"])
                # full-tile squares (partition 0 of psD0 is all-zero: LD_ev
                # col 0 is zero; partition 0 of psD1 is the finite carry-out)
                nc.scalar.square(out=cur["d20"][:, :], in_=cur["psD0"][:, :])
                nc.scalar.square(out=cur["d21"][:, :], in_=cur["psD1"][:, :])

                # ---- Vector queue: carries first (feed next TE slot), then
                # output muls of pair i-1 (no TE consumer; only out-DMA) -----
                if i + 1 < NPAIR:
                    sn0 = slice(2 * (i + 1) * LC, (2 * (i + 1) + 1) * LC)
                    nc.vector.tensor_copy(out=W[0:1, sn0], in_=cur["psD1"][0:1, :])
                # v-carry into d20[0] — emitted AFTER the full-tile square so
                # the WAW on partition 0 resolves with the carry value last
                if i == 0:
                    nc.vector.tensor_copy(out=cur["d20"][0:1, :], in_=v0_t[:])
                else:
                    nc.vector.tensor_copy(
                        out=cur["d20"][0:1, :], in_=prev["psV1"][0:1, :]
                    )
                if i > 0:
                    nc.vector.tensor_mul(
                        out=W[:, prev["s0"]], in0=prev["psD0"][:, :], in1=prev["r0"][:, :]
                    )
                    nc.vector.tensor_mul(
                        out=W[:, prev["s1"]], in0=prev["psD1"][:, :], in1=prev["r1"][:, :]
                    )
                    emit_out_dma(2 * i)  # blocks of pair i-1 now complete

                prev = cur

            # ---- drain pair 31's V phase ------------------------------------
            st = slice(64 * LC, 65 * LC)
            nc.tensor.matmul(prev["psV1"][:], LV_od, prev["d21"][:], start=True, stop=False)
            nc.tensor.matmul(prev["psV1"][:], LV_x, prev["d20"][:], start=False, stop=True)
            nc.tensor.matmul(prev["psV0"][:], LV_ev, prev["d20"][:], start=True, stop=True)
            nc.vector.tensor_copy(out=W[0:1, st], in_=prev["psD1"][0:1, :])
            rsqrt_to(prev["r0"], prev["psV0"])
            rsqrt_to(prev["r1"], prev["psV1"])
            nc.vector.tensor_mul(out=W[:, prev["s0"]], in0=prev["psD0"][:, :], in1=prev["r0"][:, :])
            nc.vector.tensor_mul(out=W[:, prev["s1"]], in0=prev["psD1"][:, :], in1=prev["r1"][:, :])
            emit_out_dma(64)

            # ---- tail: block 64 (64 rows; pretile zero-padded the rest) -----
            psDt = psD_pool.tile([128, LC], f32, tag="psD0")
            nc.tensor.matmul(psDt[:], LD_ev, W[:, st], start=True, stop=True)
            d2t = d2_pool.tile([128, LC], bf16, tag="d20")
            nc.scalar.square(out=d2t[:, :], in_=psDt[:, :])
            nc.vector.tensor_copy(out=d2t[0:1, :], in_=prev["psV1"][0:1, :])
            psVt = psV_pool.tile([128, LC], f32, tag="psV0", bufs=1)
            nc.tensor.matmul(psVt[:], LV_ev, d2t[:], start=True, stop=True)
            rt = r_pool.tile([128, LC], f32, tag="r0")
            rsqrt_to(rt, psVt)
            nc.vector.tensor_mul(out=W[:, st], in0=psDt[:, :], in1=rt[:, :])
            emit_out_dma(NBLK)

    nc.compile()
    _PROGRAM_CACHE["nc"] = nc
    return nc


def kernel(x: np.ndarray, m: np.ndarray, var: np.ndarray) -> np.ndarray:
    from concourse.bass_utils import run_bass_kernel_spmd

    x = np.asarray(x, dtype=_f32)
    m = np.ascontiguousarray(np.asarray(m, dtype=_f32))
    var = np.ascontiguousarray(np.asarray(var, dtype=_f32))
    assert x.shape == (N_ROWS, L_FULL), x.shape

    nc = _build_program()
    mats = _build_mats()

    in_maps = []
    for c in range(N_CORES):
        sl = slice(c * LC, (c + 1) * LC)
        in_maps.append(
            {
                "xp": _pretile(np.ascontiguousarray(x[:, sl]), m[sl]),
                "var": np.ascontiguousarray(var[sl]),
                "mats": mats,
            }
        )

    res = run_bass_kernel_spmd(nc, in_maps, core_ids=list(range(N_CORES)))
    out = np.concatenate(
        [_unpretile(res.results[c]["op"]) for c in range(N_CORES)], axis=1
    )
    return out.astype(_f32, copy=False)
